# revision 3
# baseline (speedup 1.0000x reference)
"""Trainium2 Bass kernel for nn_AttentionLayer_41154376630717 (sparse_attention).

Sharding: 8 cores <- (head h, row-parity r): core c handles head h = c//2 and
query rows l = r::2 (r = c%2) of BOTH batches.  phi/u are read once per (h, row)
across the machine; the (B,H,L,S) attention tensor A, entropy, and V outputs
partition cleanly along the same split.  Row-parity (rather than contiguous
halves) keeps the causal-mask block structure identical on every core so one
SPMD program serves all 8.

Math per core (L'=1024 local rows, S=2048):
  scores_raw = Q K^T              (PE, fp32, E=64 contraction)
  att0 = gelu(tanh(sc*(scores_raw + causal_NEG)))   sc = gain/tau/sqrt(E)
  logit = ln(u+1e-8) - ln(1-u+1e-8);  z = logit + phi (DMA-accumulated)
  m = sigmoid(z / tau_gs)
  A = att0 * m  (cols <= l), A = gelu(-1) * m  (masked cols, exact saturation)
  entropy = -sum_s A*ln(max(A,1e-8))  (masked cols folded as ln(1e-8)*sum(A))
  V = A @ Vmat                     (PE transposes A 128x128-blockwise)

ACT table sets are thrashy (~2.7us/switch), so work is grouped into phases per
G row-tiles: [natural_log: ln(u), ln(1-u), ln(Amax) deferred] ->
[sigmoid set: m] -> [gelu set: tanh, gelu + all PE/DVE work].
"""

import math
from contextlib import ExitStack

import numpy as np

import bass_rust
import concourse.bass as bass
import concourse.tile as tile
from concourse import mybir
from concourse.bass_utils import run_bass_kernel_spmd
from concourse.masks import make_identity

F = mybir.ActivationFunctionType
OP = mybir.AluOpType
FP32 = mybir.dt.float32

B, L, S, H, E = 2, 2048, 2048, 4, 64
NEG = -1e9
EPS_ENT = 1e-8
N_CORES = 8
LC = L // 2          # rows per core (per batch)
NT = LC // 128       # 8 row-tiles of 128
G = 2                # m row-tiles per table-set phase group
MMW = 512            # matmul free-dim chunk

# fp32 constant: gelu(-1.0) exactly as the fp32 reference computes it
C0 = float(np.float32(-0.5 * (1.0 - math.erf(1.0 / math.sqrt(2.0)))))
L0 = float(np.float32(math.log(EPS_ENT)))


def _split_multi_waits(nc, max_w=1):
    """This container's walrus rejects >1 sync-wait per instruction; split
    extra waits onto injected Drain carriers placed just before."""
    for fn in nc.m.functions:
        for bb in fn.blocks:
            insts = list(bb.instructions)
            out = []
            changed = False
            for inst in insts:
                si = inst.sync_info
                if si and si.on_wait and len(si.on_wait) > max_w:
                    waits = list(si.on_wait)
                    extra, keep = waits[:-max_w], waits[-max_w:]
                    for k in range(0, len(extra), max_w):
                        d = mybir.InstDrain(name=f"{inst.name}-w{k}")
                        d.engine = inst.engine
                        d.sync_info = bass_rust.SyncInfo(
                            on_wait=extra[k : k + max_w], on_update=[]
                        )
                        out.append(d)
                    si.on_wait = keep
                    changed = True
                out.append(inst)
            if changed:
                bb.instructions = out


def _register_const_ap(nc, value, dtype=FP32):
    if (dtype, value) in nc.const_aps.aps:
        return
    t = nc.alloc_sbuf_tensor(f"const-{dtype.name}-{value}", [128, 1], dtype)
    nc.gpsimd.memset(t.ap(), value)
    nc.const_aps.aps[(dtype, value)] = t.ap()


def _build(causal, sc, inv_tau_gs):
    nc = bass.Bass()
    _register_const_ap(nc, EPS_ENT)
    _register_const_ap(nc, 1.0 + EPS_ENT)

    qT = nc.declare_dram_parameter("qT", [B, E, LC], FP32, isOutput=False)
    kT = nc.declare_dram_parameter("kT", [B, E, S], FP32, isOutput=False)
    vm = nc.declare_dram_parameter("vm", [B, S, E], FP32, isOutput=False)
    u_s = nc.declare_dram_parameter("u_s", [LC, S], FP32, isOutput=False)
    phi_s = nc.declare_dram_parameter("phi_s", [LC, S], FP32, isOutput=False)
    maskt = nc.declare_dram_parameter("maskt", [128, 256], FP32, isOutput=False)
    A_out = nc.declare_dram_parameter("A_out", [B, LC, S], FP32, isOutput=True)
    V_out = nc.declare_dram_parameter("V_out", [B, LC, E], FP32, isOutput=True)
    ent_out = nc.declare_dram_parameter("ent_out", [B, LC], FP32, isOutput=True)

    def W_of(t):
        return 256 * (t + 1) if causal else S

    with ExitStack() as ctx:
        tc = ctx.enter_context(tile.TileContext(nc))
        singles = ctx.enter_context(tc.tile_pool(name="singles", bufs=1))
        p_u = ctx.enter_context(tc.tile_pool(name="p_u", bufs=2))
        p_t1 = ctx.enter_context(tc.tile_pool(name="p_t1", bufs=2))
        p_zm = ctx.enter_context(tc.tile_pool(name="p_zm", bufs=G + 1))
        p_ths = ctx.enter_context(tc.tile_pool(name="p_ths", bufs=2))
        p_A = ctx.enter_context(tc.tile_pool(name="p_A", bufs=2 * G + 1))
        p_at = ctx.enter_context(tc.tile_pool(name="p_at", bufs=2))
        p_esc = ctx.enter_context(tc.tile_pool(name="p_esc", bufs=2))
        p_escr = ctx.enter_context(tc.tile_pool(name="p_escr", bufs=2))
        p_small = ctx.enter_context(tc.tile_pool(name="p_small", bufs=8))
        p_vsb = ctx.enter_context(tc.tile_pool(name="p_vsb", bufs=2))
        ps_sc = ctx.enter_context(tc.tile_pool(name="ps_sc", bufs=3, space="PSUM"))
        ps_at = ctx.enter_context(tc.tile_pool(name="ps_at", bufs=2, space="PSUM"))
        ps_vt = ctx.enter_context(tc.tile_pool(name="ps_vt", bufs=2, space="PSUM"))
        ps_vn = ctx.enter_context(tc.tile_pool(name="ps_vn", bufs=1, space="PSUM"))

        ident = singles.tile([128, 128], FP32)
        make_identity(nc, ident)
        mk = singles.tile([128, 256], FP32)
        nc.sync.dma_start(out=mk, in_=maskt[:, :])
        qT_sb = []
        kT_sb = []
        vm_sb = []
        for b in range(B):
            qt = singles.tile([E, LC], FP32, tag=f"qT{b}")
            nc.sync.dma_start(out=qt, in_=qT[b])
            qT_sb.append(qt)
            kt = singles.tile([E, S], FP32, tag=f"kT{b}")
            nc.sync.dma_start(out=kt, in_=kT[b])
            kT_sb.append(kt)
            vt = singles.tile([128, S // 128, E], FP32, tag=f"vm{b}")
            nc.sync.dma_start(out=vt, in_=vm[b].rearrange("(j p) d -> p j d", p=128))
            vm_sb.append(vt)
        ent_sb = singles.tile([128, B, NT], FP32)

        pending = []  # (A_tile, sum_up | None, W, b, t)

        def flush_pending():
            # natural_log-set work deferred from the previous gelu phase
            for A_t, sum_up, W, b, t in pending:
                amax = p_esc.tile([128, S], FP32, tag="amax")
                nc.vector.tensor_scalar(
                    out=amax[:, :W], in0=A_t[:, :W], scalar1=EPS_ENT,
                    scalar2=None, op0=OP.max,
                )
                nc.scalar.activation(out=amax[:, :W], in_=amax[:, :W], func=F.Ln)
                scr = p_escr.tile([128, S], FP32, tag="escr")
                ent1 = p_small.tile([128, 1], FP32, tag="ent1")
                # scr = A * ln(Amax); ent1 = sum(scr)
                nc.vector.scalar_tensor_tensor(
                    out=scr[:, :W], in0=A_t[:, :W], scalar=0.0, in1=amax[:, :W],
                    op0=OP.bypass, op1=OP.mult, accum_out=ent1,
                )
                if sum_up is not None:
                    # ent = -L0*sum_up - ent1
                    nc.vector.scalar_tensor_tensor(
                        out=ent_sb[:, b, t : t + 1], in0=sum_up, scalar=-L0,
                        in1=ent1, op0=OP.mult, op1=OP.subtract,
                    )
                else:
                    nc.vector.tensor_scalar(
                        out=ent_sb[:, b, t : t + 1], in0=ent1, scalar1=-1.0,
                        scalar2=None, op0=OP.mult,
                    )
            pending.clear()

        for g0 in range(0, NT, G):
            group = range(g0, min(g0 + G, NT))
            # ---- phase 1: natural_log set ----
            z_tiles = {}
            for t in group:
                rows = slice(t * 128, (t + 1) * 128)
                ut = p_u.tile([128, S], FP32, tag="u")
                nc.sync.dma_start(out=ut, in_=u_s[rows, :])
                t1 = p_t1.tile([128, S], FP32, tag="t1")
                nc.scalar.activation(out=t1, in_=ut, func=F.Ln, bias=EPS_ENT)
                nc.scalar.activation(
                    out=ut, in_=ut, func=F.Ln, bias=1.0 + EPS_ENT, scale=-1.0
                )
                z = p_zm.tile([128, S], FP32, tag="zm")
                nc.vector.tensor_tensor(out=z, in0=t1, in1=ut, op=OP.subtract)
                # z += phi, accumulated inline by the DMA engines (SWDGE)
                nc.gpsimd.dma_start(out=z, in_=phi_s[rows, :], accum_op=OP.add)
                z_tiles[t] = z
            flush_pending()
            # ---- phase 2: sigmoid set ----
            for t in group:
                z = z_tiles[t]
                nc.scalar.activation(
                    out=z, in_=z, func=F.Sigmoid, scale=inv_tau_gs
                )  # z tile now holds m
            # ---- phase 3: gelu set (tanh + gelu) + PE/DVE/DMA work ----
            for t in group:
                m = z_tiles[t]
                W = W_of(t)
                U = S - W
                for b in range(B):
                    rows = slice(t * 128, (t + 1) * 128)
                    ths = p_ths.tile([128, S], FP32, tag="ths")
                    for c0q in range(0, W, MMW):
                        cw = min(MMW, W - c0q)
                        ps = ps_sc.tile([128, MMW], FP32, tag="ps")
                        nc.tensor.matmul(
                            ps[:, :cw],
                            qT_sb[b][:, rows],
                            kT_sb[b][:, c0q : c0q + cw],
                            start=True, stop=True,
                        )
                        if causal and c0q <= 256 * t < c0q + cw:
                            d0 = 256 * t - c0q
                            nc.vector.tensor_tensor(
                                out=ps[:, d0 : d0 + 256],
                                in0=ps[:, d0 : d0 + 256],
                                in1=mk, op=OP.add,
                            )
                        nc.scalar.activation(
                            out=ths[:, c0q : c0q + cw], in_=ps[:, :cw],
                            func=F.Tanh, scale=sc,
                        )
                    nc.scalar.activation(
                        out=ths[:, :W], in_=ths[:, :W], func=F.Gelu
                    )  # ths now holds att0
                    A_t = p_A.tile([128, S], FP32, tag="A")
                    nc.vector.tensor_tensor(
                        out=A_t[:, :W], in0=ths[:, :W], in1=m[:, :W], op=OP.mult
                    )
                    sum_up = None
                    if U > 0:
                        sum_up = p_small.tile([128, 1], FP32, tag="sumup")
                        nc.vector.tensor_scalar(
                            out=A_t[:, W:], in0=m[:, W:], scalar1=C0,
                            scalar2=None, op0=OP.mult, op1=OP.add,
                            accum_out=sum_up,
                        )
                    nc.sync.dma_start(out=A_out[b, rows, :], in_=A_t)
                    # A^T via PE, then V = A @ Vmat
                    at = p_at.tile([128, S], FP32, tag="at")
                    for jj in range(0, S // 128, 4):
                        atp = ps_at.tile([128, 512], FP32, tag="atp")
                        for k in range(4):
                            j = jj + k
                            nc.tensor.transpose(
                                atp[:, k * 128 : (k + 1) * 128],
                                A_t[:, j * 128 : (j + 1) * 128],
                                ident,
                            )
                        nc.vector.tensor_copy(
                            out=at[:, jj * 128 : (jj + 4) * 128], in_=atp
                        )
                    vt_ps = ps_vt.tile([E, 128], FP32, tag="vt")
                    nj = S // 128
                    for j in range(nj):
                        nc.tensor.matmul(
                            vt_ps,
                            vm_sb[b][:, j, :],
                            at[:, j * 128 : (j + 1) * 128],
                            start=(j == 0), stop=(j == nj - 1),
                        )
                    vt_sb = p_vsb.tile([E, 128], FP32, tag="vtsb")
                    nc.vector.tensor_copy(out=vt_sb, in_=vt_ps)
                    vn_ps = ps_vn.tile([128, E], FP32, tag="vn")
                    nc.tensor.transpose(vn_ps, vt_sb, ident[:E, :E])
                    vn_sb = p_vsb.tile([128, E], FP32, tag="vnsb")
                    nc.vector.tensor_copy(out=vn_sb, in_=vn_ps)
                    nc.sync.dma_start(out=V_out[b, rows, :], in_=vn_sb)
                    pending.append((A_t, sum_up, W, b, t))
        flush_pending()
        nc.sync.dma_start(
            out=ent_out[:, :].rearrange("b (t p) -> p b t", p=128), in_=ent_sb
        )

    _split_multi_waits(nc)
    return nc


_CACHE = {}


def _get_nc(causal, sc, inv_tau_gs):
    key = (causal, round(sc, 12), round(inv_tau_gs, 12))
    if key not in _CACHE:
        _CACHE[key] = _build(causal, sc, inv_tau_gs)
    return _CACHE[key]


def kernel(query, key, value, mask_miss_k, mask_miss_q, pos, causal_mask,
           phi, u, log_gain, log_tau, log_tau_gs):
    query = np.asarray(query, dtype=np.float32)
    key = np.asarray(key, dtype=np.float32)
    value = np.asarray(value, dtype=np.float32)
    phi = np.asarray(phi, dtype=np.float32)
    u = np.asarray(u, dtype=np.float32)
    causal = bool(int(np.asarray(causal_mask)))

    gain = float(np.clip(np.exp(np.float32(log_gain)), 0.001, 10.0))
    tau = float(np.clip(np.exp(np.float32(log_tau)), 0.001, 10.0))
    tau_gs = float(np.clip(np.exp(np.float32(log_tau_gs)), 0.1, 5.0))
    sc = gain / tau / math.sqrt(E)

    nc = _get_nc(causal, sc, 1.0 / tau_gs)

    p = np.arange(128)
    in_maps = []
    for c in range(N_CORES):
        h, r = c // 2, c % 2
        mt = np.where(
            np.arange(256)[None, :] <= (2 * p + r)[:, None], 0.0, NEG
        ).astype(np.float32)
        in_maps.append({
            "qT": np.ascontiguousarray(query[:, r::2, h, :].transpose(0, 2, 1)),
            "kT": np.ascontiguousarray(key[:, :, h, :].transpose(0, 2, 1)),
            "vm": np.ascontiguousarray(value[:, :, h, :]),
            "u_s": np.ascontiguousarray(u[h, r::2, :]),
            "phi_s": np.ascontiguousarray(phi[h, r::2, :]),
            "maskt": mt,
        })

    res = run_bass_kernel_spmd(nc, in_maps, list(range(N_CORES))).results

    V = np.empty((B, L, H, E), dtype=np.float32)
    A = np.empty((B, H, L, S), dtype=np.float32)
    ent = np.empty((B, H, L), dtype=np.float32)
    for c in range(N_CORES):
        h, r = c // 2, c % 2
        A[:, h, r::2, :] = res[c]["A_out"]
        V[:, r::2, h, :] = res[c]["V_out"]
        ent[:, h, r::2] = res[c]["ent_out"]
    return V, A, ent
